# revision 30
# baseline (speedup 1.0000x reference)
"""Trainium2 Bass kernel for nn_MinimalBeatDecoder (nms_detection), v3.

Reference semantics: peaks = positive local maxima of a 7-wide window over a
16.7M-frame logit stream; output = frame index of each peak (sections are
single peaks in the no-tie case), first 2^21 of them, padded with -1.

Per core (2^21 frames as 128 rows x 16384), chunks [512,1536,4096x3,1536,512]:
  - DVE computes a pair-candidate flag with just 3 ops per chunk:
      P[s]  = bf16(max(x[2s], x[2s+1], 0))  one 1x STT straight from the
              strided fp32 input (relu + bf16 cast + pair max fused; the
              rounding is monotone so the comparisons stay a SUPERSET)
      R[s]  = max(P[s-1], P[s+1])           neighbor-pair max (2x)
      pk[s] = P[s] >= R[s]                  pair holds the 6-window max (2x)
    (pk == (max-of-pair >= 6-window max) because P >= P trivially; any
    pair containing a true 7-window peak must satisfy it.)
  - the flags stream straight back to DRAM (no on-device compaction: the
    only compaction engine, GPSIMD LocalScatter, runs at ~3ns/idx and
    starves the DVE while active, costing more than it saves).
  - host: flagged pairs -> candidate = pair argmax (one fp32 compare),
    then exact fp32 verification of every candidate (vectorized 7-window
    max at candidate positions) removes the ~15% false candidates.

An exact numpy fallback handles inputs with adjacent-equal fp32 peak ties
(reference merges those into averaged sections; gaussian inputs never tie).
"""

import sys

sys.path.insert(0, "/opt/trn_rl_repo")

import numpy as np

import concourse.bacc as bacc
import concourse.bass as bass
import concourse.mybir as mybir
import concourse.tile as tile
from concourse import bass_utils

# geometry
NCORES = 8
NFRAMES = 16_777_216
PERCORE = NFRAMES // NCORES  # 2^21
MAX_BEATS = NFRAMES // 8  # 2^21

P = 128
W = PERCORE // P  # 16384 frames per row
WP = W // 2  # 8192 pairs per row
HALO = 8

CHUNKS = [512, 1536, 4096, 4096, 4096, 1536, 512]

F32 = mybir.dt.float32
BF16 = mybir.dt.bfloat16
I16 = mybir.dt.int16
I8 = mybir.dt.int8


def build_kernel(p=P, w=W):
    """Per-core SPMD program. Inputs:
      xin [p*w + HALO] f32  (frame t of this core at index t+4)
    Outputs:
      mk [p, WP] i16  (pair-candidate flags)
    """
    nc = bacc.Bacc("TRN2", target_bir_lowering=False)
    xin = nc.dram_tensor("xin", [p * w + HALO], F32, kind="ExternalInput")
    mk_d = nc.dram_tensor("mk", [p, WP], I16, kind="ExternalOutput")

    MX = mybir.AluOpType.max
    GE = mybir.AluOpType.is_ge
    RELU = mybir.ActivationFunctionType.Relu

    with tile.TileContext(nc) as tc:
        with (
            tc.tile_pool(name="io", bufs=7) as io_pool,
            tc.tile_pool(name="wk", bufs=3) as wk_pool,
        ):
            o = 0  # pair offset within row
            for j, cw in enumerate(CHUNKS):
                hw = cw // 2
                off = 2 * o
                xh = io_pool.tile([p, 4104], F32, tag="xh")
                src = bass.AP(tensor=xin, offset=off, ap=[[w, p], [1, cw + 8]])
                nc.scalar.dma_start(xh[:, 0 : cw + 8], src)

                # pair max with relu folded, straight from fp32 (one 1x STT;
                # bf16 out is monotone: bf16(max(a,b)) == max(bf16 a, bf16 b))
                Pt = wk_pool.tile([p, 2052], BF16, tag="Pt")
                nc.vector.scalar_tensor_tensor(
                    out=Pt[:, 0 : hw + 4], in0=xh[:, 0 : cw + 8 : 2],
                    scalar=0.0, in1=xh[:, 1 : cw + 8 : 2],
                    op0=MX, op1=MX,
                )
                Rt = wk_pool.tile([p, 2048], BF16, tag="Rt")
                nc.vector.tensor_tensor(
                    out=Rt[:, 0:hw], in0=Pt[:, 1 : hw + 1],
                    in1=Pt[:, 3 : hw + 3], op=MX,
                )
                # pair-candidate flag: the pair's max beats both neighbor
                # pairs <=> it is the 6-window max. This is a SUPERSET of
                # "pair contains a true 7-window peak"; the host recovers
                # the in-pair parity from fp32 x and verifies exactly.
                pk = wk_pool.tile([p, 2048], I16, tag="pk")
                nc.vector.tensor_tensor(
                    out=pk[:, 0:hw], in0=Pt[:, 2 : hw + 2], in1=Rt[:, 0:hw],
                    op=GE,
                )

                nc.sync.dma_start(mk_d[:, o : o + hw], pk[:, 0:hw])
                o += hw
    nc.compile()
    return nc


_cached = {}


def _get_nc():
    if "nc" not in _cached:
        _cached["nc"] = build_kernel()
    return _cached["nc"]


def _host_reference_fallback(x):
    """Exact numpy fallback (used only for adjacent-equal fp32 peak ties)."""
    n = x.shape[0]
    import numpy.lib.stride_tricks as st

    xp = np.pad(x, (3, 3), constant_values=-np.inf)
    pooled = st.sliding_window_view(xp, 7).max(axis=1)
    peak = (x == pooled) & (x > 0)
    idx = np.arange(n, dtype=np.int64)
    prev = np.concatenate([[False], peak[:-1]])
    is_new = peak & ~prev
    sec = np.cumsum(is_new) - 1
    sums = np.zeros(MAX_BEATS + 1, np.float64)
    cnts = np.zeros(MAX_BEATS + 1, np.float64)
    sel = peak & (sec < MAX_BEATS)
    np.add.at(sums, sec[sel], idx[sel].astype(np.float64))
    np.add.at(cnts, sec[sel], 1.0)
    out = np.full(MAX_BEATS, -1.0, np.float32)
    m = cnts[:MAX_BEATS] > 0
    out[m] = (sums[:MAX_BEATS][m] / cnts[:MAX_BEATS][m]).astype(np.float32)
    return out[None, :]


def kernel(logit: np.ndarray) -> np.ndarray:
    x = np.asarray(logit, dtype=np.float32)[0]

    # host guard: adjacent-equal fp32 window maxima need the exact path
    eq_next = x[:-1] == x[1:]
    if eq_next.any():
        cand = np.nonzero(eq_next)[0]
        cand = cand[(x[cand] > 0)]
        if cand.size:
            xp = np.pad(x, (3, 3), constant_values=-np.inf)
            for i in cand:
                if (
                    x[i] == xp[i : i + 7].max()
                    and x[i + 1] == xp[i + 1 : i + 8].max()
                ):
                    return _host_reference_fallback(x)

    nc = _get_nc()

    xpad = np.full(NFRAMES + 8, np.float32(-3.0e38), dtype=np.float32)
    xpad[4 : 4 + NFRAMES] = x

    in_maps = []
    for c in range(NCORES):
        base = c * PERCORE
        in_maps.append(
            {"xin": np.ascontiguousarray(xpad[base : base + PERCORE + HALO])}
        )

    global _last_in_maps
    _last_in_maps = in_maps
    res = bass_utils.run_bass_kernel_spmd(nc, in_maps, core_ids=list(range(NCORES)))

    # host: flagged pairs -> candidate positions (pair max, parity from
    # fp32 x; flat pair index == global pair since row-major == frame order)
    mk = np.concatenate([res.results[c]["mk"].reshape(-1) for c in range(NCORES)])
    nz = np.flatnonzero(mk)
    even_pos = 2 * nz
    parity = x[even_pos + 1] > x[even_pos]
    cand = even_pos + parity

    # exact fp32 verification of every candidate (removes bf16/relu ties)
    xg = np.pad(x, (3, 3), constant_values=-np.float32(np.inf))
    win = xg[cand[:, None] + np.arange(7)[None, :]]
    xv = x[cand]
    keep = (xv >= win.max(axis=1)) & (xv > 0)
    beats = cand[keep][:MAX_BEATS]

    out = np.full(MAX_BEATS, -1.0, dtype=np.float32)
    out[: beats.size] = beats.astype(np.float32)
    return out[None, :]


# revision 31
# speedup vs baseline: 1.1317x; 1.1317x over previous
"""Trainium2 Bass kernel for nn_MinimalBeatDecoder (nms_detection), v3.

Reference semantics: peaks = positive local maxima of a 7-wide window over a
16.7M-frame logit stream; output = frame index of each peak (sections are
single peaks in the no-tie case), first 2^21 of them, padded with -1.

Per core (2^21 frames as 128 rows x 16384), chunks [512,1536,4096x3,1536,512]:
  - DVE computes a pair-candidate flag with just 3 ops per chunk:
      P[s]  = bf16(max(x[2s], x[2s+1], 0))  one 1x STT straight from the
              strided fp32 input (relu + bf16 cast + pair max fused; the
              rounding is monotone so the comparisons stay a SUPERSET)
      R[s]  = max(P[s-1], P[s+1])           neighbor-pair max (2x)
      pk[s] = P[s] >= R[s]                  pair holds the 6-window max (2x)
    (pk == (max-of-pair >= 6-window max) because P >= P trivially; any
    pair containing a true 7-window peak must satisfy it.)
  - the flags stream straight back to DRAM (no on-device compaction: the
    only compaction engine, GPSIMD LocalScatter, runs at ~3ns/idx and
    starves the DVE while active, costing more than it saves).
  - host: flagged pairs -> candidate = pair argmax (one fp32 compare),
    then exact fp32 verification of every candidate (vectorized 7-window
    max at candidate positions) removes the ~15% false candidates.

An exact numpy fallback handles inputs with adjacent-equal fp32 peak ties
(reference merges those into averaged sections; gaussian inputs never tie).
"""

import sys

sys.path.insert(0, "/opt/trn_rl_repo")

import numpy as np

import concourse.bacc as bacc
import concourse.bass as bass
import concourse.mybir as mybir
import concourse.tile as tile
from concourse import bass_utils

# geometry
NCORES = 8
NFRAMES = 16_777_216
PERCORE = NFRAMES // NCORES  # 2^21
MAX_BEATS = NFRAMES // 8  # 2^21

P = 128
W = PERCORE // P  # 16384 frames per row
WP = W // 2  # 8192 pairs per row
HALO = 8

CHUNKS = [512, 1536, 4096, 4096, 4096, 1536, 512]

F32 = mybir.dt.float32
BF16 = mybir.dt.bfloat16
I16 = mybir.dt.int16
I8 = mybir.dt.int8


def build_kernel(p=P, w=W):
    """Per-core SPMD program. Inputs:
      xin [p*w + HALO] f32  (frame t of this core at index t+4)
    Outputs:
      mk [p, WP] i16  (pair-candidate flags)
    """
    nc = bacc.Bacc("TRN2", target_bir_lowering=False)
    xin = nc.dram_tensor("xin", [p * w + HALO], F32, kind="ExternalInput")
    mk_d = nc.dram_tensor("mk", [p, WP], I16, kind="ExternalOutput")

    MX = mybir.AluOpType.max
    GE = mybir.AluOpType.is_ge
    RELU = mybir.ActivationFunctionType.Relu

    with tile.TileContext(nc) as tc:
        with (
            tc.tile_pool(name="io", bufs=6) as io_pool,
            tc.tile_pool(name="wk", bufs=3) as wk_pool,
        ):
            o = 0  # pair offset within row
            for j, cw in enumerate(CHUNKS):
                hw = cw // 2
                off = 2 * o
                xh = io_pool.tile([p, 4104], F32, tag="xh")
                src = bass.AP(tensor=xin, offset=off, ap=[[w, p], [1, cw + 8]])
                nc.scalar.dma_start(xh[:, 0 : cw + 8], src)

                # pair max with relu folded, straight from fp32 (one 1x STT;
                # bf16 out is monotone: bf16(max(a,b)) == max(bf16 a, bf16 b))
                Pt = wk_pool.tile([p, 2052], BF16, tag="Pt")
                nc.vector.scalar_tensor_tensor(
                    out=Pt[:, 0 : hw + 4], in0=xh[:, 0 : cw + 8 : 2],
                    scalar=0.0, in1=xh[:, 1 : cw + 8 : 2],
                    op0=MX, op1=MX,
                )
                Rt = wk_pool.tile([p, 2048], BF16, tag="Rt")
                nc.vector.tensor_tensor(
                    out=Rt[:, 0:hw], in0=Pt[:, 1 : hw + 1],
                    in1=Pt[:, 3 : hw + 3], op=MX,
                )
                # pair-candidate flag: the pair's max beats both neighbor
                # pairs <=> it is the 6-window max. This is a SUPERSET of
                # "pair contains a true 7-window peak"; the host recovers
                # the in-pair parity from fp32 x and verifies exactly.
                pk = wk_pool.tile([p, 2048], I16, tag="pk")
                nc.vector.tensor_tensor(
                    out=pk[:, 0:hw], in0=Pt[:, 2 : hw + 2], in1=Rt[:, 0:hw],
                    op=GE,
                )

                nc.sync.dma_start(mk_d[:, o : o + hw], pk[:, 0:hw])
                o += hw
    nc.compile()
    return nc


_cached = {}


def _get_nc():
    if "nc" not in _cached:
        _cached["nc"] = build_kernel()
    return _cached["nc"]


def _host_reference_fallback(x):
    """Exact numpy fallback (used only for adjacent-equal fp32 peak ties)."""
    n = x.shape[0]
    import numpy.lib.stride_tricks as st

    xp = np.pad(x, (3, 3), constant_values=-np.inf)
    pooled = st.sliding_window_view(xp, 7).max(axis=1)
    peak = (x == pooled) & (x > 0)
    idx = np.arange(n, dtype=np.int64)
    prev = np.concatenate([[False], peak[:-1]])
    is_new = peak & ~prev
    sec = np.cumsum(is_new) - 1
    sums = np.zeros(MAX_BEATS + 1, np.float64)
    cnts = np.zeros(MAX_BEATS + 1, np.float64)
    sel = peak & (sec < MAX_BEATS)
    np.add.at(sums, sec[sel], idx[sel].astype(np.float64))
    np.add.at(cnts, sec[sel], 1.0)
    out = np.full(MAX_BEATS, -1.0, np.float32)
    m = cnts[:MAX_BEATS] > 0
    out[m] = (sums[:MAX_BEATS][m] / cnts[:MAX_BEATS][m]).astype(np.float32)
    return out[None, :]


def kernel(logit: np.ndarray) -> np.ndarray:
    x = np.asarray(logit, dtype=np.float32)[0]

    # host guard: adjacent-equal fp32 window maxima need the exact path
    eq_next = x[:-1] == x[1:]
    if eq_next.any():
        cand = np.nonzero(eq_next)[0]
        cand = cand[(x[cand] > 0)]
        if cand.size:
            xp = np.pad(x, (3, 3), constant_values=-np.inf)
            for i in cand:
                if (
                    x[i] == xp[i : i + 7].max()
                    and x[i + 1] == xp[i + 1 : i + 8].max()
                ):
                    return _host_reference_fallback(x)

    nc = _get_nc()

    xpad = np.full(NFRAMES + 8, np.float32(-3.0e38), dtype=np.float32)
    xpad[4 : 4 + NFRAMES] = x

    in_maps = []
    for c in range(NCORES):
        base = c * PERCORE
        in_maps.append(
            {"xin": np.ascontiguousarray(xpad[base : base + PERCORE + HALO])}
        )

    global _last_in_maps
    _last_in_maps = in_maps
    res = bass_utils.run_bass_kernel_spmd(nc, in_maps, core_ids=list(range(NCORES)))

    # host: flagged pairs -> candidate positions (pair max, parity from
    # fp32 x; flat pair index == global pair since row-major == frame order)
    mk = np.concatenate([res.results[c]["mk"].reshape(-1) for c in range(NCORES)])
    nz = np.flatnonzero(mk)
    even_pos = 2 * nz
    parity = x[even_pos + 1] > x[even_pos]
    cand = even_pos + parity

    # exact fp32 verification of every candidate (removes bf16/relu ties)
    xg = np.pad(x, (3, 3), constant_values=-np.float32(np.inf))
    win = xg[cand[:, None] + np.arange(7)[None, :]]
    xv = x[cand]
    keep = (xv >= win.max(axis=1)) & (xv > 0)
    beats = cand[keep][:MAX_BEATS]

    out = np.full(MAX_BEATS, -1.0, dtype=np.float32)
    out[: beats.size] = beats.astype(np.float32)
    return out[None, :]
